# revision 19
# baseline (speedup 1.0000x reference)
"""Trainium2 Bass kernel for nn_DilatedMask: 33x33 binary mask dilation.

Computes, for x of shape (8, 2048, 2048, 1) float32 with values in a raster
where NODATA == 0.0:
    mask = (x == 0)
    y    = sliding-window max of mask over a 33x33 window (SAME padding),
           as uint8.

Strategy (per NeuronCore, pure data parallel over the batch of 8):
  A square max window over a binary mask equals (2D box-sum of mask) > 0,
  and the box sum is separable.  Both 1-D 33-wide box sums are computed on
  the TensorEngine as banded matmuls.  Using the image tile as the
  *stationary* operand makes each pass transpose its output, so pass 1
  (H-axis sum) emits a transposed intermediate [w, h] and pass 2 (W-axis
  sum over that) lands back in natural [h, w] orientation -- no explicit
  transposes anywhere.

    mask  = is_equal(x, 0)                       (DVE, f32 -> bf16)
    S1^T  = band^T-sum over H of mask, out [w,h] (PE, banded matmul)
    S1b   = copy/cast S1^T to bf16               (ACT, PSUM -> SBUF)
    S2    = band-sum over W of S1b, out [h,w]    (PE, banded matmul)
    y     = (S2 > 0.5) as uint8                  (DVE, PSUM -> SBUF)

PSUM accumulation: per 512-col PSUM bank the first matmul piece uses
start=True (bank reset), the last uses stop=True, everything between
accumulates (start=False), exploiting per-element has_written bits.
"""

from contextlib import ExitStack

import numpy as np
import ml_dtypes

RADIUS = 16
SE = 2 * RADIUS + 1  # 33
P = 128
BANDW = P + 2 * RADIUS  # 160: out-columns reachable from one 128-row k-tile
BANK = 512  # PSUM bank width in f32 elements
H = W = 2048
N_CORES = 8


def band_np() -> np.ndarray:
    """Band matrix chunk [128, 160] bf16.

    band[p, j] = 1  iff  the out column (h_out = 128*kt - RADIUS + j) is
    within RADIUS of input row (h' = 128*kt + p):  j - 2*RADIUS <= p <= j.
    """
    p = np.arange(P)[:, None]
    j = np.arange(BANDW)[None, :]
    return ((p <= j) & (p >= j - 2 * RADIUS)).astype(ml_dtypes.bfloat16)


def _split_at_banks(lo: int, hi: int):
    """Split [lo, hi) at multiples of BANK."""
    out = []
    while lo < hi:
        nxt = min(hi, (lo // BANK + 1) * BANK)
        out.append((lo, nxt))
        lo = nxt
    return out


def _pieces_for_pass(n: int):
    """All matmul pieces for one banded-sum pass with n output columns.

    Returns list of (kt, lo, hi, start, stop): k-tile index, out-column
    range, and PSUM start/stop flags.  Emission order is ascending kt;
    per PSUM bank exactly the first piece has start=True and the last has
    stop=True.
    """
    nt = n // P
    raw = []  # (kt, lo, hi)
    for kt in range(nt):
        base = P * kt - RADIUS
        fresh_lo = 0 if kt == 0 else P * kt + RADIUS
        fresh_hi = min(n, P * kt + P + RADIUS)
        for lo, hi in _split_at_banks(fresh_lo, fresh_hi):
            raw.append((kt, lo, hi))
        if kt > 0:
            for lo, hi in _split_at_banks(P * kt - RADIUS, P * kt + RADIUS):
                raw.append((kt, lo, hi))
    first_in_bank = {}
    last_in_bank = {}
    for i, (kt, lo, hi) in enumerate(raw):
        b = lo // BANK
        if b not in first_in_bank:
            first_in_bank[b] = i
        last_in_bank[b] = i
    return [
        (kt, lo, hi, i == first_in_bank[lo // BANK], i == last_in_bank[lo // BANK])
        for i, (kt, lo, hi) in enumerate(raw)
    ]


def build_program(h: int = H, w: int = W):
    """Build the per-core Bass program (SPMD, identical on all cores)."""
    import concourse.bass as bass
    import concourse.mybir as mybir
    import concourse.tile as tile
    from concourse import bacc

    f32 = mybir.dt.float32
    bf16 = mybir.dt.bfloat16
    fp8 = mybir.dt.float8e4
    u8 = mybir.dt.uint8

    nt_h = h // P
    nt_w = w // P

    nc = bacc.Bacc("TRN2", target_bir_lowering=False, debug=False)
    x_ap = nc.dram_tensor("x", [h, w], f32, kind="ExternalInput").ap()
    band8_ap = nc.dram_tensor("band8", [P, BANDW], fp8, kind="ExternalInput").ap()
    y_ap = nc.dram_tensor("y", [h, w], u8, kind="ExternalOutput").ap()

    # x row strips are loaded in groups of GRP strips per DMA (SWDGE w/ inline
    # f32->bf16 cast).  Walrus allows at most ONE sync-wait on DMA/Ldweights
    # instructions, so every staged tile gets a dedicated slot (bufs = count)
    # to avoid slot-reuse WAR/WAW waits, and output stores are grouped so each
    # of the 8 HWDGE lanes is used at most once.
    GRP = 4
    n_grp = max(1, nt_h // GRP)
    grp = nt_h // n_grp
    OGRP = 2
    n_ogrp = max(1, nt_h // OGRP)
    ogrp = nt_h // n_ogrp

    with tile.TileContext(nc) as tc, ExitStack() as ctx:
        band_pool = ctx.enter_context(tc.tile_pool(name="band", bufs=1))
        raw_pool = ctx.enter_context(tc.tile_pool(name="raw", bufs=n_grp))
        m_pool = ctx.enter_context(tc.tile_pool(name="m", bufs=n_grp))
        s1_pool = ctx.enter_context(tc.tile_pool(name="s1", bufs=nt_w))
        ps_pool = ctx.enter_context(tc.tile_pool(name="ps", bufs=2, space="PSUM"))
        out_pool = ctx.enter_context(tc.tile_pool(name="out", bufs=n_ogrp))

        band8_t = band_pool.tile([P, BANDW], fp8, tag="band8")
        nc.gpsimd.dma_start(out=band8_t[:], in_=band8_ap[:, :])

        # Persistent PSUM tiles: reusing the same memref keeps the PE's
        # write-after-write on PSUM in program order (one sem wait budget).
        ps_tiles = [
            ps_pool.tile([P, max(h, w)], f32, tag="ps", name=f"ps{i}")
            for i in range(2)
        ]
        n_ps = 0

        def next_ps():
            nonlocal n_ps
            t = ps_tiles[n_ps % 2]
            n_ps += 1
            return t

        # Load row-strip groups (cast f32->bf16 in DMA), then mask:
        # m_grp[g] [128, grp*w] fp8 = (x == 0) for rows [g*grp*128, ...)
        m_grps = []
        for g in range(n_grp):
            raw = raw_pool.tile([P, grp * w], bf16)
            src = x_ap[g * grp * P : (g + 1) * grp * P, :].rearrange(
                "(a p) w -> p a w", p=P
            )
            dst = raw[:].rearrange("p (a w) -> p a w", a=grp)
            nc.gpsimd.dma_start(out=dst, in_=src)
            m = m_pool.tile([P, grp * w], fp8)
            nc.vector.tensor_scalar(
                m[:], raw[:], 0.0, None, mybir.AluOpType.is_equal
            )
            m_grps.append(m)

        def m_stat(kt: int, wt: int):
            g, a = divmod(kt, grp)
            off = a * w + wt * P
            return m_grps[g][:, off : off + P]

        pieces_h = _pieces_for_pass(h)
        pieces_w = _pieces_for_pass(w)

        # Pass 1: H-axis band sum; output transposed strips S1^T[wt] [w', h]
        s1_tiles = []
        for wt in range(nt_w):
            ps1 = next_ps()
            for kt, lo, hi, st, sp in pieces_h:
                base = P * kt - RADIUS
                nc.tensor.matmul(
                    ps1[:, lo:hi],
                    m_stat(kt, wt),
                    band8_t[:, lo - base : hi - base],
                    start=st,
                    stop=sp,
                )
            # Evacuate + binarize: s1 = sign(count) in {0,1}, fp8 to save SBUF
            s1 = s1_pool.tile([P, h], fp8)
            nc.scalar.sign(s1[:, :], ps1[:, :h])
            s1_tiles.append(s1)

        # Pass 2: W-axis band sum over S1^T; output natural [h, w].
        # Output rows are grouped OGRP strips per store DMA (HWDGE, one lane
        # each); both thresholds of a group run on the same engine so the
        # store needs a single sync wait.  Thresholds alternate DVE/ACT per
        # group for load balance.
        for og in range(n_ogrp):
            yt = out_pool.tile([P, ogrp * w], u8)
            for a in range(ogrp):
                ht = og * ogrp + a
                ps2 = next_ps()
                for wt, lo, hi, st, sp in pieces_w:
                    base = P * wt - RADIUS
                    nc.tensor.matmul(
                        ps2[:, lo:hi],
                        s1_tiles[wt][:, ht * P : (ht + 1) * P],
                        band8_t[:, lo - base : hi - base],
                        start=st,
                        stop=sp,
                    )
                if og % 2 == 0:
                    nc.vector.tensor_scalar(
                        yt[:, a * w : (a + 1) * w], ps2[:, :w], 0.5, None,
                        mybir.AluOpType.is_gt,
                    )
                else:
                    nc.scalar.sign(yt[:, a * w : (a + 1) * w], ps2[:, :w])
            dst = y_ap[og * ogrp * P : (og + 1) * ogrp * P, :].rearrange(
                "(a p) w -> p a w", p=P
            )
            nc.sync.dma_start(
                out=dst, in_=yt[:].rearrange("p (a w) -> p a w", a=ogrp)
            )

    nc.compile()
    return nc


def kernel(x: np.ndarray) -> np.ndarray:
    """Full-input entry point: x (8, 2048, 2048, 1) f32 -> y same shape uint8."""
    from concourse.bass_utils import run_bass_kernel_spmd

    x = np.asarray(x)
    assert x.shape == (N_CORES, H, W, 1), x.shape
    imgs = np.ascontiguousarray(x[:, :, :, 0], dtype=np.float32)

    nc = build_program(H, W)
    band8 = band_np().astype(ml_dtypes.float8_e4m3)
    in_maps = [{"x": imgs[c], "band8": band8} for c in range(N_CORES)]
    res = run_bass_kernel_spmd(nc, in_maps, list(range(N_CORES)))
    y = np.stack([res.results[c]["y"] for c in range(N_CORES)])
    return y[..., None]


# revision 22
# speedup vs baseline: 1.0696x; 1.0696x over previous
"""Trainium2 Bass kernel for nn_DilatedMask: 33x33 binary mask dilation.

Computes, for x of shape (8, 2048, 2048, 1) float32 with values in a raster
where NODATA == 0.0:
    mask = (x == 0)
    y    = sliding-window max of mask over a 33x33 window (SAME padding),
           as uint8.

Strategy (per NeuronCore, pure data parallel over the batch of 8):
  A square max window over a binary mask equals (2D box-sum of mask) > 0,
  and the box sum is separable.  Both 1-D 33-wide box sums are computed on
  the TensorEngine as banded matmuls.  Using the image tile as the
  *stationary* operand makes each pass transpose its output, so pass 1
  (H-axis sum) emits a transposed intermediate [w, h] and pass 2 (W-axis
  sum over that) lands back in natural [h, w] orientation -- no explicit
  transposes anywhere.

    mask  = is_equal(x, 0)                       (DVE, f32 -> bf16)
    S1^T  = band^T-sum over H of mask, out [w,h] (PE, banded matmul)
    S1b   = copy/cast S1^T to bf16               (ACT, PSUM -> SBUF)
    S2    = band-sum over W of S1b, out [h,w]    (PE, banded matmul)
    y     = (S2 > 0.5) as uint8                  (DVE, PSUM -> SBUF)

PSUM accumulation: per 512-col PSUM bank the first matmul piece uses
start=True (bank reset), the last uses stop=True, everything between
accumulates (start=False), exploiting per-element has_written bits.
"""

from contextlib import ExitStack

import numpy as np
import ml_dtypes

RADIUS = 16
SE = 2 * RADIUS + 1  # 33
P = 128
BANDW = P + 2 * RADIUS  # 160: out-columns reachable from one 128-row k-tile
BANK = 512  # PSUM bank width in f32 elements
H = W = 2048
N_CORES = 8


def band_np() -> np.ndarray:
    """Band matrix chunk [128, 160] bf16.

    band[p, j] = 1  iff  the out column (h_out = 128*kt - RADIUS + j) is
    within RADIUS of input row (h' = 128*kt + p):  j - 2*RADIUS <= p <= j.
    """
    p = np.arange(P)[:, None]
    j = np.arange(BANDW)[None, :]
    return ((p <= j) & (p >= j - 2 * RADIUS)).astype(ml_dtypes.bfloat16)


def _split_at_banks(lo: int, hi: int):
    """Split [lo, hi) at multiples of BANK."""
    out = []
    while lo < hi:
        nxt = min(hi, (lo // BANK + 1) * BANK)
        out.append((lo, nxt))
        lo = nxt
    return out


def _pieces_for_pass(n: int):
    """All matmul pieces for one banded-sum pass with n output columns.

    Returns list of (kt, lo, hi, start, stop): k-tile index, out-column
    range, and PSUM start/stop flags.  Emission order is ascending kt;
    per PSUM bank exactly the first piece has start=True and the last has
    stop=True.
    """
    nt = n // P
    raw = []  # (kt, lo, hi)
    for kt in range(nt):
        base = P * kt - RADIUS
        fresh_lo = 0 if kt == 0 else P * kt + RADIUS
        fresh_hi = min(n, P * kt + P + RADIUS)
        for lo, hi in _split_at_banks(fresh_lo, fresh_hi):
            raw.append((kt, lo, hi))
        if kt > 0:
            for lo, hi in _split_at_banks(P * kt - RADIUS, P * kt + RADIUS):
                raw.append((kt, lo, hi))
    first_in_bank = {}
    last_in_bank = {}
    for i, (kt, lo, hi) in enumerate(raw):
        b = lo // BANK
        if b not in first_in_bank:
            first_in_bank[b] = i
        last_in_bank[b] = i
    return [
        (kt, lo, hi, i == first_in_bank[lo // BANK], i == last_in_bank[lo // BANK])
        for i, (kt, lo, hi) in enumerate(raw)
    ]


def build_program(h: int = H, w: int = W):
    """Build the per-core Bass program (SPMD, identical on all cores)."""
    import concourse.bass as bass
    import concourse.mybir as mybir
    import concourse.tile as tile
    from concourse import bacc

    f32 = mybir.dt.float32
    bf16 = mybir.dt.bfloat16
    fp8 = mybir.dt.float8e4
    u8 = mybir.dt.uint8

    nt_h = h // P
    nt_w = w // P

    nc = bacc.Bacc("TRN2", target_bir_lowering=False, debug=False)
    x_ap = nc.dram_tensor("x", [h, w], f32, kind="ExternalInput").ap()
    band8_ap = nc.dram_tensor("band8", [P, BANDW], fp8, kind="ExternalInput").ap()
    y_ap = nc.dram_tensor("y", [h, w], u8, kind="ExternalOutput").ap()

    OGRP = 2
    n_ogrp = max(1, nt_h // OGRP)
    ogrp = nt_h // n_ogrp

    with tile.TileContext(nc) as tc, ExitStack() as ctx:
        band_pool = ctx.enter_context(tc.tile_pool(name="band", bufs=1))
        xf_pool = ctx.enter_context(tc.tile_pool(name="xf", bufs=4))
        m_pool = ctx.enter_context(tc.tile_pool(name="m", bufs=4))
        s1_pool = ctx.enter_context(tc.tile_pool(name="s1", bufs=nt_w))
        ps_pool = ctx.enter_context(tc.tile_pool(name="ps", bufs=2, space="PSUM"))
        out_pool = ctx.enter_context(tc.tile_pool(name="out", bufs=n_ogrp))

        band8_t = band_pool.tile([P, BANDW], fp8, tag="band8")
        nc.gpsimd.dma_start(out=band8_t[:], in_=band8_ap[:, :])

        # Persistent PSUM tiles: reusing the same memref keeps the PE's
        # write-after-write on PSUM in program order (one sem wait budget).
        ps_tiles = [
            ps_pool.tile([P, max(h, w)], f32, tag="ps", name=f"ps{i}")
            for i in range(2)
        ]
        n_ps = 0

        def next_ps():
            nonlocal n_ps
            t = ps_tiles[n_ps % 2]
            n_ps += 1
            return t

        pieces_h = _pieces_for_pass(h)
        pieces_w = _pieces_for_pass(w)

        # Pass 1 with pipelined column-strip loads: for each 128-wide column
        # strip wt, DMA x[:, wt] (f32, HWDGE), mask it (DVE, f32->fp8), then
        # run the H-axis banded matmuls for that strip.  PE starts after the
        # first strip instead of after the whole image.
        # Strip layout: xcol[p, kt*128 + c] = x[kt*128 + p, wt*128 + c].
        s1_tiles = []
        for wt in range(nt_w):
            xf = xf_pool.tile([P, h], f32)
            src = x_ap[:, wt * P : (wt + 1) * P].rearrange(
                "(kt p) c -> p kt c", p=P
            )
            nc.sync.dma_start(
                out=xf[:].rearrange("p (kt c) -> p kt c", kt=nt_h), in_=src
            )
            m = m_pool.tile([P, h], fp8)
            nc.vector.tensor_scalar(
                m[:], xf[:], 0.0, None, mybir.AluOpType.is_equal
            )

            ps1 = next_ps()
            for kt, lo, hi, st, sp in pieces_h:
                base = P * kt - RADIUS
                nc.tensor.matmul(
                    ps1[:, lo:hi],
                    m[:, kt * P : (kt + 1) * P],
                    band8_t[:, lo - base : hi - base],
                    start=st,
                    stop=sp,
                )
            # Evacuate + binarize: s1 = sign(count) in {0,1}, fp8 to save
            # SBUF.  Alternate DVE/ACT for engine balance.
            s1 = s1_pool.tile([P, h], fp8)
            if wt % 2 == 0:
                nc.scalar.sign(s1[:, :], ps1[:, :h])
            else:
                nc.vector.tensor_scalar(
                    s1[:, :], ps1[:, :h], 0.5, None, mybir.AluOpType.is_gt
                )
            s1_tiles.append(s1)

        # Pass 2: W-axis band sum over S1^T; output natural [h, w].
        # Output rows are grouped OGRP strips per store DMA (HWDGE, one lane
        # each); both thresholds of a group run on the same engine so the
        # store needs a single sync wait.  Thresholds alternate DVE/ACT per
        # group for load balance.
        for og in range(n_ogrp):
            yt = out_pool.tile([P, ogrp * w], u8)
            for a in range(ogrp):
                ht = og * ogrp + a
                ps2 = next_ps()
                for wt, lo, hi, st, sp in pieces_w:
                    base = P * wt - RADIUS
                    nc.tensor.matmul(
                        ps2[:, lo:hi],
                        s1_tiles[wt][:, ht * P : (ht + 1) * P],
                        band8_t[:, lo - base : hi - base],
                        start=st,
                        stop=sp,
                    )
                if og % 4 == 0:
                    nc.vector.tensor_scalar(
                        yt[:, a * w : (a + 1) * w], ps2[:, :w], 0.5, None,
                        mybir.AluOpType.is_gt,
                    )
                else:
                    nc.scalar.sign(yt[:, a * w : (a + 1) * w], ps2[:, :w])
            dst = y_ap[og * ogrp * P : (og + 1) * ogrp * P, :].rearrange(
                "(a p) w -> p a w", p=P
            )
            nc.sync.dma_start(
                out=dst, in_=yt[:].rearrange("p (a w) -> p a w", a=ogrp)
            )

    nc.compile()
    return nc


def kernel(x: np.ndarray) -> np.ndarray:
    """Full-input entry point: x (8, 2048, 2048, 1) f32 -> y same shape uint8."""
    from concourse.bass_utils import run_bass_kernel_spmd

    x = np.asarray(x)
    assert x.shape == (N_CORES, H, W, 1), x.shape
    imgs = np.ascontiguousarray(x[:, :, :, 0], dtype=np.float32)

    nc = build_program(H, W)
    band8 = band_np().astype(ml_dtypes.float8_e4m3)
    in_maps = [{"x": imgs[c], "band8": band8} for c in range(N_CORES)]
    res = run_bass_kernel_spmd(nc, in_maps, list(range(N_CORES)))
    y = np.stack([res.results[c]["y"] for c in range(N_CORES)])
    return y[..., None]
